# revision 23
# baseline (speedup 1.0000x reference)
"""BERT-embedding kernel for 8 Trainium2 NeuronCores (Bass/Tile).

out[b,s,:] = concat( input[b,s,:] @ W.T + b_vec,  PE[doy[b,s], :] )
with PE the standard sinusoidal table (d_model=256, max_len=366).

Strategy (data-parallel over batch, 8 cores):
  - core c handles batches [c*128, (c+1)*128) = 16384 tokens.
  - obs half: TensorE matmul. Two token tiles are packed per matmul with a
    block-diagonal stationary operand (K = 2*11 = 22, N = 512) so one
    LDWEIGHTS covers two tiles. Matmul runs in float32r (TF32-like),
    ~4x the fp32 moving rate; obs-half error ~6e-4 of absmax.
  - PE half: computed on the fly: sin/cos(doy*div[i]) on the ACT engine.
    The Sin spline is only valid on [-pi, pi], so angles for the
    low-frequency columns i < R are range-reduced with the f32
    magic-number rounding trick on DVE; for i >= R the angle is already
    < pi since doy < 366. cos(y) = sin(pi/2 - |y|) with |y| from ACT Abs.
  - output assembled in SBUF as [128 tokens, 512] tiles, streamed out with
    large (4 MB) HWDGE DMAs. The kernel is HBM-write-bound (~33.5 MB/core).
"""
import numpy as np

# ---------------- problem constants (hardcoded per contract) ----------------
B, S, F, D = 1024, 128, 10, 256
MAX_LEN = 366
N_CORES = 8
BPC = B // N_CORES          # batches per core
TOK = BPC * S               # tokens per core = 16384
P = 128                     # tokens per tile (SBUF partitions)
G = TOK // P                # 128 tiles per core
# group sizes (tiles per group): small leading groups so the first output
# DMAs fire early, then steady 8-tile groups
GROUP_PLAN = [2, 2, 4, 8] + [8] * 14
assert sum(GROUP_PLAN) == G
K = F + 1                   # contraction dim incl. bias row
K2 = 2 * K                  # packed two-tile contraction dim
R = 68                      # columns needing range reduction (365*div[68] < pi)

MM_MODE = "f32r"            # "f32r" (fast, ~6e-4) or "f32" (exact, slower PE)

PI = float(np.float32(np.pi))
HALF_PI = float(np.float32(np.pi / 2))
TWO_PI = float(np.float32(2 * np.pi))
INV_2PI = float(np.float32(1.0 / (2 * np.pi)))
MAGIC = 12582912.0          # 1.5 * 2**23: (x+MAGIC)-MAGIC == round-to-nearest(x)

# of the 64 pair copies (PSUM->SBUF), route this many to ACT, rest to DVE
ACT_COPY_RATIO = (14, 64)

_CACHE = {}


def _pair_on_act(pair_idx):
    num, den = ACT_COPY_RATIO
    return (pair_idx * num) % den < num


def _build_nc():
    import concourse.bacc as bacc
    import concourse.tile as tile
    import concourse.mybir as mybir

    F32 = mybir.dt.float32
    F32R = mybir.dt.float32r
    AOT = mybir.AluOpType
    ACT = mybir.ActivationFunctionType

    mm_dt = F32R if MM_MODE == "f32r" else F32

    nc = bacc.Bacc("TRN2", target_bir_lowering=False, debug=False,
                   num_devices=N_CORES)
    # lhs packs token-tile pairs: [2K, TOK/2]; col pair*P+p holds tile 2*pair
    # token p's features in rows 0:K and tile 2*pair+1 token p's in rows K:2K.
    lhs_d = nc.dram_tensor("lhs", [K2, TOK // 2], mm_dt, kind="ExternalInput")
    # rhs block-diagonal [2K, 2D]: [[R,0],[0,R]]
    rhs_d = nc.dram_tensor("rhsw", [K2, 2 * D], mm_dt, kind="ExternalInput")
    doy_d = nc.dram_tensor("doyT", [P, G], F32, kind="ExternalInput")
    div_d = nc.dram_tensor("divb", [P, 128], F32, kind="ExternalInput")
    out_d = nc.dram_tensor("out", [TOK, 2 * D], F32, kind="ExternalOutput")

    # out rows viewed as (t, p): row = t*P + p
    outv = out_d[:].rearrange("(t p) c -> p t c", p=P)

    with tile.TileContext(nc) as tc:
        with (
            tc.tile_pool(name="const", bufs=1) as cpool,
            tc.tile_pool(name="angp", bufs=4) as angp,
            tc.tile_pool(name="outp", bufs=5) as outp,
            tc.tile_pool(name="psum", bufs=6, space="PSUM") as psump,
        ):
            doy_sb = cpool.tile([P, G], F32)
            nc.sync.dma_start(doy_sb[:], doy_d[:])
            div_sb = cpool.tile([P, 128], F32)
            nc.sync.dma_start(div_sb[:], div_d[:])
            HEAD = 8  # leading pairs (16 tiles) in their own tile
            lt_head = cpool.tile([K2, HEAD * P], mm_dt)
            nc.sync.dma_start(lt_head[:], lhs_d[:, 0:HEAD * P])
            rhs_sb = cpool.tile([K2, 2 * D], mm_dt)
            nc.sync.dma_start(rhs_sb[:], rhs_d[:])
            halfpi = cpool.tile([P, 1], F32)
            nc.vector.memset(halfpi[:], HALF_PI)
            # rest of lhs resident: removes per-group input DMAs from the
            # steady state so all queues stream output. Two pieces so the
            # first full-size groups are not gated on the whole transfer.
            MID = 24
            lt_mid = cpool.tile([K2, MID * P], mm_dt)
            nc.sync.dma_start(lt_mid[:], lhs_d[:, HEAD * P:(HEAD + MID) * P])
            lt_tail = cpool.tile([K2, (G // 2 - HEAD - MID) * P], mm_dt)
            nc.sync.dma_start(lt_tail[:], lhs_d[:, (HEAD + MID) * P:])

            t0 = 0
            pair0 = 0
            for tpg in GROUP_PLAN:
                npair = tpg // 2
                pl, ph = t0 // 2, (t0 + tpg) // 2
                if ph <= HEAD:
                    lt = lt_head[:, pl * P:ph * P]
                elif ph <= HEAD + MID:
                    lt = lt_mid[:, (pl - HEAD) * P:(ph - HEAD) * P]
                else:
                    lt = lt_tail[:, (pl - HEAD - MID) * P:(ph - HEAD - MID) * P]

                og = outp.tile([P, tpg, 2 * D], F32, tag="og")
                tg = angp.tile([P, tpg, 128], F32, tag="tg")

                # tg[p,t,i] = doy[p, t0+t] * div[i]
                div_b = (
                    div_sb[:].rearrange("p i -> p () i").to_broadcast([P, tpg, 128])
                )
                doy_b = (
                    doy_sb[:, t0:t0 + tpg]
                    .rearrange("p t -> p t ()")
                    .to_broadcast([P, tpg, 128])
                )
                nc.vector.tensor_tensor(out=tg[:], in0=div_b, in1=doy_b, op=AOT.mult)

                # range-reduce cols < R into [-pi, pi]:
                #   q = round(t/2pi);  t -= 2pi*q
                uc = angp.tile([P, tpg, R], F32, tag="uc")
                nc.vector.tensor_scalar(
                    out=uc[:], in0=tg[:, :, 0:R], scalar1=INV_2PI, scalar2=MAGIC,
                    op0=AOT.mult, op1=AOT.add,
                )
                nq = angp.tile([P, tpg, R], F32, tag="nq")
                nc.vector.tensor_scalar(
                    out=nq[:], in0=uc[:], scalar1=MAGIC, scalar2=-TWO_PI,
                    op0=AOT.subtract, op1=AOT.mult,
                )
                nc.vector.tensor_tensor(
                    out=tg[:, :, 0:R], in0=tg[:, :, 0:R], in1=nq[:], op=AOT.add
                )
                # |y| over all cols for the cos identity cos(y) = sin(pi/2 - |y|)
                ay = angp.tile([P, tpg, 128], F32, tag="ay")
                nc.scalar.activation(ay[:], tg[:], ACT.Abs)

                # obs half: one matmul per token-tile pair (block-diag pack)
                for p2 in range(npair):
                    ps = psump.tile([P, 2 * D], F32, tag="ps")
                    nc.tensor.matmul(
                        ps[:], lt[:, p2 * P:(p2 + 1) * P], rhs_sb[:]
                    )
                    src = ps[:].rearrange("p (t c) -> p t c", t=2)
                    dst = og[:, 2 * p2:2 * p2 + 2, 0:D]
                    if _pair_on_act(pair0 + p2):
                        nc.scalar.copy(dst, src)
                    else:
                        nc.vector.tensor_copy(out=dst, in_=src)

                # PE half: interleaved sin/cos via ACT
                nc.scalar.activation(og[:, :, D::2], tg[:], ACT.Sin)
                nc.scalar.activation(
                    og[:, :, D + 1::2], ay[:], ACT.Sin,
                    scale=-1.0, bias=halfpi[:],
                )

                nc.sync.dma_start(outv[:, t0:t0 + tpg, :], og[:])
                t0 += tpg
                pair0 += npair
    nc.compile()
    return nc


def _host_prep(input_sequence, doy_sequence, W, b):
    x = np.ascontiguousarray(np.asarray(input_sequence, dtype=np.float32))
    doy = np.asarray(doy_sequence)
    Wf = np.asarray(W, dtype=np.float32)
    bf = np.asarray(b, dtype=np.float32)

    # block-diagonal rhs [2K, 2D]
    rhs = np.zeros((K2, 2 * D), dtype=np.float32)
    rhs[:F, :D] = Wf.T
    rhs[F, :D] = bf
    rhs[K:K + F, D:] = Wf.T
    rhs[K + F, D:] = bf

    div = np.exp(
        np.arange(0, D, 2, dtype=np.float32) * np.float32(-np.log(10000.0) / D)
    ).astype(np.float32)
    divb = np.broadcast_to(div, (P, D // 2)).copy()

    xs = x.reshape(N_CORES, TOK, F)
    ds = doy.reshape(N_CORES, TOK).astype(np.float32)

    in_maps = []
    for c in range(N_CORES):
        # packed lhs: [2K, TOK/2]; tiles interleaved pairwise
        xt = xs[c].reshape(G, P, F)          # [tile, p, f]
        lhs = np.zeros((K2, TOK // 2), dtype=np.float32)
        xt_even = xt[0::2]                   # [G/2, P, F]
        xt_odd = xt[1::2]
        # cols: pair-major then p
        lhs[:F] = xt_even.transpose(2, 0, 1).reshape(F, TOK // 2)
        lhs[F] = 1.0
        lhs[K:K + F] = xt_odd.transpose(2, 0, 1).reshape(F, TOK // 2)
        lhs[K + F] = 1.0
        doyT = np.ascontiguousarray(ds[c].reshape(G, P).T)
        in_maps.append({"lhs": lhs, "rhsw": rhs, "doyT": doyT, "divb": divb})
    return in_maps


def _get_nc():
    if "nc" not in _CACHE:
        _CACHE["nc"] = _build_nc()
    return _CACHE["nc"]


def kernel(input_sequence, doy_sequence, W, b, _trace=False, _trace_kwargs=None):
    from concourse.bass_utils import run_bass_kernel_spmd

    nc = _get_nc()
    in_maps = _host_prep(input_sequence, doy_sequence, W, b)
    kw = {}
    if _trace:
        kw.update(trace=True, **(_trace_kwargs or {}))
    res = run_bass_kernel_spmd(nc, in_maps, core_ids=list(range(N_CORES)), **kw)
    out = np.concatenate([res.results[c]["out"] for c in range(N_CORES)], axis=0)
    out = out.reshape(B, S, 2 * D)
    if _trace:
        _CACHE["last_results"] = res
    return out
